# revision 13
# baseline (speedup 1.0000x reference)
import os
import numpy as np
import ml_dtypes

BF = ml_dtypes.bfloat16
B, S, HID, NH, D = 64, 512, 256, 4, 64
NCORES = 8
BPC = B // NCORES  # batches per core

_cache = {}


def _hilo(bias):
    b32 = bias.astype(np.float32).reshape(HID)
    hi = b32.astype(BF)
    lo = (b32 - hi.astype(np.float32)).astype(BF)
    return np.stack([hi, lo])


def _build_nc():
    from concourse import bacc, tile
    from concourse import bass
    mybir = bass.mybir
    dt = mybir.dt
    af = mybir.ActivationFunctionType
    alu = mybir.AluOpType

    nc = bacc.Bacc("TRN2", target_bir_lowering=False, debug=False,
                   enable_asserts=True, num_devices=NCORES)

    bf, f32 = dt.bfloat16, dt.float32
    qT_d = nc.declare_dram_parameter("qT", [BPC, 2, 128, 512], bf, isOutput=False)
    kT_d = nc.declare_dram_parameter("kT", [BPC, 2, 128, 512], bf, isOutput=False)
    vT_d = nc.declare_dram_parameter("vT", [BPC, 2, 128, 512], bf, isOutput=False)
    mk_d = nc.declare_dram_parameter("mk", [BPC, 4, 128, 512], bf, isOutput=False)
    Wq_d = nc.declare_dram_parameter("Wq", [2, 128, 256], bf, isOutput=False)
    Wk_d = nc.declare_dram_parameter("Wk", [2, 128, 256], bf, isOutput=False)
    Wv_d = nc.declare_dram_parameter("Wv", [2, 128, 64], bf, isOutput=False)
    Wh_d = nc.declare_dram_parameter("Wh", [64, 256], bf, isOutput=False)
    bq_d = nc.declare_dram_parameter("bq", [2, 256], bf, isOutput=False)
    bk_d = nc.declare_dram_parameter("bk", [2, 256], bf, isOutput=False)
    bvb_d = nc.declare_dram_parameter("bvb", [128, 64], f32, isOutput=False)
    bhb_d = nc.declare_dram_parameter("bhb", [128, 256], f32, isOutput=False)
    iq_d = nc.declare_dram_parameter("iq", [128, 128], bf, isOutput=False)
    i32_d = nc.declare_dram_parameter("i32", [128, 128], f32, isOutput=False)
    att_d = nc.declare_dram_parameter("attention", [BPC, 4, 128, 512], f32, isOutput=True)
    out_d = nc.declare_dram_parameter("output", [BPC, 4, 128, 256], f32, isOutput=True)

    with tile.TileContext(nc) as tc:
        with (
            tc.tile_pool(name="wpool", bufs=1) as wpool,
            tc.tile_pool(name="inpool", bufs=2) as inpool,
            tc.tile_pool(name="xpool", bufs=2) as xpool,
            tc.tile_pool(name="ppool", bufs=2) as ppool,
            tc.tile_pool(name="apool", bufs=2) as apool,
            tc.tile_pool(name="spool", bufs=4) as spool,
            tc.tile_pool(name="pepool", bufs=1, space="PSUM") as pepool,
            tc.tile_pool(name="pspool", bufs=4, space="PSUM") as pspool,
        ):
            # --- weights / constants (once) ---
            wq = wpool.tile([128, 2, 256], bf, name="wq")
            nc.sync.dma_start(wq[:], Wq_d[:].transpose([1, 0, 2]))
            wk = wpool.tile([128, 2, 256], bf, name="wk")
            nc.sync.dma_start(wk[:], Wk_d[:].transpose([1, 0, 2]))
            wv = wpool.tile([128, 2, 64], bf, name="wv")
            nc.sync.dma_start(wv[:], Wv_d[:].transpose([1, 0, 2]))
            wh = wpool.tile([64, 256], bf, name="wh")
            nc.sync.dma_start(wh[:], Wh_d[:])
            bqr = wpool.tile([2, 256], bf, name="bqr")
            nc.sync.dma_start(bqr[:], bq_d[:])
            bkr = wpool.tile([2, 256], bf, name="bkr")
            nc.sync.dma_start(bkr[:], bk_d[:])
            ones = wpool.tile([2, 512], bf, name="ones")
            nc.vector.memset(ones[:], 1.0)
            bvb = wpool.tile([128, 64], f32, name="bvb")
            nc.sync.dma_start(bvb[:], bvb_d[:])
            bhb = wpool.tile([128, 256], f32, name="bhb")
            nc.sync.dma_start(bhb[:], bhb_d[:])
            iq = wpool.tile([128, 128], bf, name="iq")
            nc.sync.dma_start(iq[:], iq_d[:])
            i32 = wpool.tile([128, 128], f32, name="i32")
            nc.sync.dma_start(i32[:], i32_d[:])

            for b in range(BPC):
                qt = inpool.tile([128, 2, 512], bf, name="qt")
                nc.sync.dma_start(qt[:], qT_d[b].transpose([1, 0, 2]))
                kt = inpool.tile([128, 2, 512], bf, name="kt")
                nc.sync.dma_start(kt[:], kT_d[b].transpose([1, 0, 2]))
                vt = inpool.tile([128, 2, 512], bf, name="vt")
                nc.sync.dma_start(vt[:], vT_d[b].transpose([1, 0, 2]))
                mk = inpool.tile([128, 4, 512], bf, name="mk")
                nc.sync.dma_start(mk[:], mk_d[b].transpose([1, 0, 2]))

                # --- projections: QT/KT halves [128,512] (feat, seq) ---
                proj_out = []
                for (w, brow, src, nm) in ((wq, bqr, qt, "q"), (wk, bkr, kt, "k")):
                    for half in range(2):
                        sl = slice(half * 128, (half + 1) * 128)
                        pp = pspool.tile([128, 512], f32, name="ps")
                        nc.tensor.matmul(pp[:], w[:, 0, sl],
                                         src[:, 0, :], start=True, stop=False)
                        nc.tensor.matmul(pp[:], w[:, 1, sl],
                                         src[:, 1, :], start=False, stop=False)
                        nc.tensor.matmul(pp[:], brow[:, sl], ones[:],
                                         start=False, stop=True)
                        dst = xpool.tile([128, 512], bf, name=f"{nm}T{half}")
                        nc.scalar.activation(dst[:], pp[:], af.Copy)
                        proj_out.append(dst)
                QTt, QTb, KTt, KTb = proj_out

                # --- V = value @ Wv + bv, layout [s(part), sc, d] ---
                vsb = xpool.tile([128, 4, 64], bf, name="vsb")
                for sc in range(4):
                    pv = pspool.tile([128, 64], f32, name="ps")
                    nc.tensor.matmul(pv[:], vt[:, 0, sc * 128:(sc + 1) * 128],
                                     wv[:, 0, :], start=True, stop=False)
                    nc.tensor.matmul(pv[:], vt[:, 1, sc * 128:(sc + 1) * 128],
                                     wv[:, 1, :], start=False, stop=True)
                    nc.vector.tensor_add(vsb[:, sc, :], pv[:], bvb[:])

                attb = apool.tile([128, 4, 512], f32, name="attb")
                attT = apool.tile([128, 4, 4, 128], bf, name="attT")

                for qc in range(4):
                    # E[q,k] for 4 heads side by side
                    pe = pepool.tile([128, 2048], f32, name="pe")
                    for h in range(4):
                        qsrc = QTt if h < 2 else QTb
                        ksrc = KTt if h < 2 else KTb
                        off = (h % 2) * 64
                        nc.tensor.matmul(pe[:, h * 512:(h + 1) * 512],
                                         qsrc[off:off + 64, qc * 128:(qc + 1) * 128],
                                         ksrc[off:off + 64, :],
                                         start=True, stop=True)
                    psb = ppool.tile([128, 2048], bf, name="psb")
                    nc.scalar.activation(psb[:], pe[:], af.Exp, scale=0.125)
                    # NOTE: fused tensor_tensor_reduce crashes on HW; use mul+reduce
                    pm = ppool.tile([128, 2048], bf, name="pm")
                    s4h = [spool.tile([128, 1], f32, name=f"s4h{h}")
                           for h in range(4)]
                    for h in range(4):
                        nc.vector.tensor_mul(pm[:, h * 512:(h + 1) * 512],
                                             psb[:, h * 512:(h + 1) * 512],
                                             mk[:, qc, :])
                        nc.vector.tensor_reduce(s4h[h][:],
                                                pm[:, h * 512:(h + 1) * 512],
                                                mybir.AxisListType.X, alu.add)
                    pa = pspool.tile([128, 512], f32, name="ps")
                    for h in range(4):
                        c4h = spool.tile([128, 1], f32, name=f"c4h{h}")
                        nc.vector.reciprocal(c4h[:], s4h[h][:])
                        dg = spool.tile([128, 128], bf, name="dg")
                        nc.vector.tensor_scalar_mul(dg[:], iq[:], c4h[:])
                        nc.tensor.matmul(pa[:], dg[:], pm[:, h * 512:(h + 1) * 512],
                                         start=(h == 0), stop=(h == 3))
                    nc.vector.tensor_copy(attb[:, qc, :], pa[:])
                    pt = pspool.tile([128, 512], f32, name="ps")
                    for kc in range(4):
                        nc.tensor.transpose(pt[:, kc * 128:(kc + 1) * 128],
                                            attb[:, qc, kc * 128:(kc + 1) * 128],
                                            i32[:])
                    nc.vector.tensor_copy(attT[:, qc, :, :], pt[:])

                # --- O^T = V^T @ att^T  [64(d), 512(q)] ---
                pOT = pspool.tile([64, 512], f32, name="ps")
                for kc in range(4):
                    nc.tensor.matmul(pOT[:], vsb[:, kc, :], attT[:, :, kc, :],
                                     start=(kc == 0), stop=(kc == 3))
                OTsb = apool.tile([64, 512], bf, name="OTsb")
                nc.scalar.activation(OTsb[:], pOT[:], af.Copy)

                # --- result = O @ Wh + bh ---
                Rb = apool.tile([128, 4, 256], f32, name="Rb")
                for qc in range(4):
                    pR = pspool.tile([128, 256], f32, name="ps")
                    nc.tensor.matmul(pR[:], OTsb[:, qc * 128:(qc + 1) * 128], wh[:],
                                     start=True, stop=True)
                    nc.vector.tensor_add(Rb[:, qc, :], pR[:], bhb[:])

                nc.sync.dma_start(att_d[b].transpose([1, 0, 2]), attb[:])
                nc.sync.dma_start(out_d[b].transpose([1, 0, 2]), Rb[:])

    nc.compile()
    return nc


def kernel(**inputs):
    from concourse.bass_utils import run_bass_kernel_spmd

    if "nc" not in _cache:
        _cache["nc"] = _build_nc()
    nc = _cache["nc"]

    q, k, v = inputs["query"], inputs["key"], inputs["value"]
    mask = inputs["mask"]
    Wq, bq = inputs["Wq"], inputs["bq"]
    Wk, bk = inputs["Wk"], inputs["bk"]
    Wv, bv = inputs["Wv"], inputs["bv"]
    Wh, bh = inputs["Wh"], inputs["bh"]

    def xt(a):  # [B,S,H] -> [B, 2, 128, S] bf16 (transposed per batch)
        t = np.ascontiguousarray(a.transpose(0, 2, 1)).astype(BF)
        return t.reshape(B, 2, 128, S)

    qT = xt(q)
    kT = xt(k)
    vT = xt(v)
    mk = mask[:, 0].astype(BF).reshape(B, 4, 128, S)

    shared = {
        "Wq": Wq.astype(BF).reshape(2, 128, HID),
        "Wk": Wk.astype(BF).reshape(2, 128, HID),
        "Wv": Wv.astype(BF).reshape(2, 128, D),
        "Wh": Wh.astype(BF),
        "bq": _hilo(bq),
        "bk": _hilo(bk),
        "bvb": np.broadcast_to(bv.astype(np.float32), (128, D)).copy(),
        "bhb": np.broadcast_to(bh.astype(np.float32), (128, HID)).copy(),
        "iq": (0.25 * np.eye(128)).astype(BF),
        "i32": np.eye(128, dtype=np.float32),
    }
    in_maps = []
    for c in range(NCORES):
        sl = slice(c * BPC, (c + 1) * BPC)
        in_maps.append({
            "qT": qT[sl], "kT": kT[sl], "vT": vT[sl], "mk": mk[sl],
            **shared,
        })

    res = run_bass_kernel_spmd(nc, in_maps, core_ids=list(range(NCORES)),
                               tmpdir=os.environ.get("BASS_TMPDIR"))
    _cache["last_result"] = res
    att = np.concatenate(
        [res.results[c]["attention"].reshape(BPC, S, S) for c in range(NCORES)], axis=0)
    out = np.concatenate(
        [res.results[c]["output"].reshape(BPC, S, HID) for c in range(NCORES)], axis=0)
    return out.astype(np.float32), att.astype(np.float32)
